# revision 31
# baseline (speedup 1.0000x reference)
"""Causal self-attention (B=4, T=2048, C=1024, H=16, D=64) on 8 trn2 NeuronCores.

Sharding: tensor-parallel over heads. Each core owns 2 heads:
  - computes Q^T/K^T/V for its heads from the (host-pretransposed) full x^T,
  - causal attention (transposed-S flash-style, softmax denominator via an
    augmented ones-column on V),
  - partial output projection with its 128 rows of W_proj.
Host sums the 8 partial projections and adds (b_v @ W_proj + b_proj).

The per-core program is identical (SPMD); only the weight-slice inputs differ.

v2 changes vs baseline:
  - causal mask applied as an extra -1e9 matmul into the diagonal S^T block
    (PE) instead of a gpsimd multiply after exp.
  - exp uses bias=-2 and writes at_sb in fp8e4m3 (faster AV weight loads,
    half the SBUF footprint).
  - AV accumulates 4 q-blocks per PSUM bank; one reciprocal per 4 blocks.
  - strips and AV interleaved per head so the scalar engine never starves.
  - qt/kt PSUM->SBUF copies moved off the scalar engine to DVE.
  - projection output copied/DMAed as bf16 (halves output traffic).
"""

import numpy as np
import ml_dtypes

import concourse.bass as bass
import concourse.bacc as bacc
import concourse.mybir as mybir
import concourse.tile as tile

B, T, C, H, D = 4, 2048, 1024, 16, 64
NCORES = 8
HPC = H // NCORES  # heads per core = 2
P = 128
NB = T // P  # 16 blocks of 128 per sequence
CK = C // P  # 8 contraction chunks for the projections

import os

F32 = mybir.dt.float32
F8 = mybir.dt.float8e4
BF16 = mybir.dt.bfloat16
AT_DT = F8 if os.environ.get("AT_FP8", "0") == "1" else BF16
ADD = mybir.AluOpType.add
MULT = mybir.AluOpType.mult
EXP = mybir.ActivationFunctionType.Exp
COPY = mybir.ActivationFunctionType.Copy


def _segments():
    """Per q-step c: the S^T strips that become complete at step c.
    Main strip (c, c, min(c+8,16)); tail strip (c-8, c, 16) for c>=8."""
    steps = []
    for c in range(NB):
        segs = [(c, c, min(c + 8, NB))]
        if c >= 8:
            segs.append((c - 8, c, NB))
        steps.append(segs)
    return steps


STEPS = _segments()


def _at_offsets():
    off = {}
    cur = 0
    for segs in STEPS:
        for (c, jlo, jhi) in segs:
            for j in range(jlo, jhi):
                off[(c, j)] = cur + (j - jlo) * P
            cur += (jhi - jlo) * P
    return off, cur


AT_OFF, AT_W = _at_offsets()  # AT_W = 136*128 = 17408


def attention_body(tc, outs, ins):
    """Tile kernel body. outs/ins are dicts of bass.APs (DRAM)."""
    nc = tc.nc
    xt = ins["xt"]  # [C, B*T] bf16 (x transposed, col = b*T + t)
    wq = ins["wq"]  # [C, 128] bf16
    wk = ins["wk"]  # [C, 128] bf16
    wv = ins["wv"]  # [C, 128] bf16
    wp = ins["wp"]          # [128, C] bf16
    bq = ins["bq"]          # [128, 1] f32 (prescaled by 0.125)
    bk = ins["bk"]          # [128, 1] f32
    maskadd = ins["maskadd"]  # [128, 128] bf16: -1e9 above diagonal (q<k)
    ident = ins["ident"]    # [128, 128] bf16 identity
    out = outs["out"]       # [B*T, C] bf16 partial projection output

    with (
        tc.tile_pool(name="consts", bufs=1) as consts,
        tc.tile_pool(name="xtp", bufs=4) as xtp,
        tc.tile_pool(name="qkp", bufs=3) as qkp,
        tc.tile_pool(name="vp", bufs=3) as vp,
        tc.tile_pool(name="atp", bufs=2) as atp,
        tc.tile_pool(name="smallp", bufs=4) as smallp,
        tc.tile_pool(name="outp", bufs=4) as outp,
        tc.tile_pool(name="pp", bufs=3, space="PSUM") as pp,
        tc.tile_pool(name="ppA", bufs=2, space="PSUM") as ppA,
        tc.tile_pool(name="ppV", bufs=1, space="PSUM") as ppV,
    ):
        # ---- constants ----
        wq_sb = consts.tile([P, CK, P], BF16, name="wq_sb")
        nc.sync.dma_start(wq_sb, wq.rearrange("(o p) m -> p o m", p=P))
        wk_sb = consts.tile([P, CK, P], BF16, name="wk_sb")
        nc.sync.dma_start(wk_sb, wk.rearrange("(o p) m -> p o m", p=P))
        wv_sb = consts.tile([P, CK, P], BF16, name="wv_sb")
        nc.sync.dma_start(wv_sb, wv.rearrange("(o p) m -> p o m", p=P))
        wp_bf = consts.tile([P, C], BF16, name="wp_bf")
        nc.sync.dma_start(wp_bf, wp)
        bq_sb = consts.tile([P, 1], F32, name="bq_sb")
        nc.gpsimd.dma_start(bq_sb, bq)
        bk_sb = consts.tile([P, 1], F32, name="bk_sb")
        nc.gpsimd.dma_start(bk_sb, bk)
        mask_sb = consts.tile([P, P], BF16, name="mask_sb")
        nc.gpsimd.dma_start(mask_sb, maskadd)
        id_sb = consts.tile([P, P], BF16, name="id_sb")
        nc.gpsimd.dma_start(id_sb, ident)
        ebias = consts.tile([P, 1], F32, name="ebias")
        nc.vector.memset(ebias, -2.0)

        keepalive = os.environ.get("KEEPALIVE", "0") == "1"

        def ka(n=2):
            # dependency-free LDWEIGHTS pulses keep the PE array active during
            # micro-stalls so the HAM clock gate stays at full rate
            if keepalive:
                for _ in range(n):
                    nc.tensor.ldweights(id_sb[:, 0:32])

        if os.environ.get("KEEPWARM", "1") == "1":
            # warm-up pre-roll: a continuous burst of dummy matmuls during the
            # initial DMA latency window un-throttles the HAM clock gate
            # before the first real stream starts (needs no DMA'd inputs)
            jw = consts.tile([P, P], BF16, name="jw")
            nc.vector.memset(jw, 0.0)
            jps = pp.tile([P, P], F32, tag="mm", name="jps")
            for _ in range(100):
                nc.tensor.matmul(jps, lhsT=jw, rhs=jw, start=True, stop=True)

        for b in range(B):
            # ======== QKV phase: Q^T, K^T (bf16), natural V (bf16, +ones col) ====
            qt = qkp.tile([P, T], BF16, tag="qt", name=f"qt_{b}")
            kt = qkp.tile([P, T], BF16, tag="kt", name=f"kt_{b}")
            vaug = vp.tile([P, NB, HPC, D + 1], BF16, tag="vaug", name=f"vaug_{b}")
            nc.gpsimd.memset(vaug[:, :, :, D:], 1.0)  # softmax-denominator column

            at_sbs = [
                atp.tile([P, AT_W], AT_DT, tag="at", name=f"at_{b}_{h}")
                for h in range(HPC)
            ]

            def do_strip(h, c, jlo, jhi):
                hs = h * D
                w = (jhi - jlo) * P
                sps = ppA.tile([P, 1024], F32, tag="sA", name=f"sps_{b}_{h}_{c}_{jlo}")
                lhs_k = kt[hs : hs + D, c * P : (c + 1) * P]
                col = 0
                while col < w:
                    n = min(512, w - col)
                    nc.tensor.matmul(
                        sps[:, col : col + n],
                        lhsT=lhs_k,
                        rhs=qt[hs : hs + D, jlo * P + col : jlo * P + col + n],
                        start=True, stop=True,
                        skip_group_check=True,
                    )
                    col += n
                if jlo == c:  # diagonal: additive causal mask via PE
                    nc.tensor.matmul(
                        sps[:, 0:P], lhsT=mask_sb, rhs=id_sb,
                        start=False, stop=True, skip_group_check=True,
                    )
                o = AT_OFF[(c, jlo)]
                nc.scalar.activation(at_sbs[h][:, o : o + w], sps[:, :w], EXP, bias=ebias)

            # steps whose qt/kt inputs are complete after quarter q4
            STEPS_AFTER_Q = {1: STEPS[0:1], 2: STEPS[1:5], 3: STEPS[5:NB]}

            for q4 in range(4):  # quarters of T (512 cols each)
                lo = q4 * 512
                xq = xtp.tile([P, CK, 512], BF16, tag="xq", name=f"xq_{b}_{q4}")
                nc.gpsimd.dma_start(
                    xq, xt[:, b * T + lo : b * T + lo + 512].rearrange("(o p) t -> p o t", p=P)
                )
                # V^T quarter first: its DVE cast hides behind the Q/K matmuls
                ps_v = pp.tile([P, 512], F32, tag="mm", name=f"psv_{b}_{q4}")
                for cc in range(CK):
                    nc.tensor.matmul(
                        ps_v, lhsT=wv_sb[:, cc], rhs=xq[:, cc],
                        start=(cc == 0), stop=(cc == CK - 1),
                    )
                vt = vp.tile([P, 512], BF16, tag="vt", name=f"vt_{b}_{q4}")
                # high priority: this cast gates the V transposes on the PE;
                # don't let it queue behind terminal attT/proj copies on DVE
                with tc.high_priority():
                    nc.vector.tensor_copy(vt, ps_v)
                # Q^T quarter
                ps_q = pp.tile([P, 512], F32, tag="mm", name=f"psq_{b}_{q4}")
                for cc in range(CK):
                    nc.tensor.matmul(
                        ps_q, lhsT=wq_sb[:, cc], rhs=xq[:, cc],
                        start=(cc == 0), stop=(cc == CK - 1),
                    )
                # qt = psum * (1/sqrt(D)) + bq_prescaled  (DVE)
                with tc.high_priority():
                    nc.vector.tensor_scalar(
                        qt[:, lo : lo + 512], ps_q, 0.125, bq_sb, MULT, ADD
                    )
                # transpose V^T -> natural V chunks (vt ready by now)
                # one accumulation group per head: mixing row-base 0/64
                # transposes in one PSUM group faults on hardware
                for h in range(HPC):
                    vtp = pp.tile([P, 4, D], BF16, tag="mm", name=f"vtp_{b}_{q4}_{h}")
                    for t4 in range(4):
                        nc.tensor.matmul(
                            vtp[:, t4],
                            lhsT=vt[h * D : (h + 1) * D, t4 * P : (t4 + 1) * P],
                            rhs=id_sb[h * D : (h + 1) * D, h * D : (h + 1) * D],
                            is_transpose=True,
                            start=(t4 == 0), stop=(t4 == 3),
                        )
                    nc.scalar.copy(vaug[:, q4 * 4 : (q4 + 1) * 4, h, 0:D], vtp)
                # K^T quarter
                ps_k = pp.tile([P, 512], F32, tag="mm", name=f"psk_{b}_{q4}")
                for cc in range(CK):
                    nc.tensor.matmul(
                        ps_k, lhsT=wk_sb[:, cc], rhs=xq[:, cc],
                        start=(cc == 0), stop=(cc == CK - 1),
                    )
                with tc.high_priority():
                    nc.vector.tensor_scalar(
                        kt[:, lo : lo + 512], ps_k, bk_sb, None, ADD
                    )
                # S^T strips whose qt/kt inputs are now complete (both heads)
                for segs in STEPS_AFTER_Q.get(q4, []):
                    for (c, jlo, jhi) in segs:
                        for h in range(HPC):
                            do_strip(h, c, jlo, jhi)
                ka()

            # ======== attention per head ========
            attT = qkp.tile([P, T], BF16, tag="attT", name=f"attT_{b}")
            att_j = [
                smallp.tile([P, HPC * D], BF16, tag="attj", bufs=20, name=f"attj_{b}_{j}")
                for j in range(NB)
            ]
            # ---- AV phase: 4 q-blocks share one PSUM bank ----
            for h in range(HPC):
                hs = h * D
                at_sb = at_sbs[h]
                for j in range(NB):
                    jj = j % 4
                    if jj == 0:
                        avq = ppV.tile([P, 4, D + 1], F32, tag="avq", name=f"avq_{b}_{h}_{j}")
                    for c in range(j + 1):
                        o = AT_OFF[(c, j)]
                        nc.tensor.matmul(
                            avq[:, jj],
                            lhsT=at_sb[:, o : o + P],
                            rhs=vaug[:, c, h],
                            start=(c == 0), stop=(c == j),
                        )
                    if jj == 3:
                        r4 = smallp.tile([P, 4], F32, tag="r4", name=f"r4_{b}_{h}_{j}")
                        nc.vector.reciprocal(r4, avq[:, :, D])
                        for j2 in range(j - 3, j + 1):
                            nc.vector.tensor_scalar(
                                att_j[j2][:, hs : hs + D], avq[:, j2 % 4, 0:D],
                                r4[:, j2 % 4 : j2 % 4 + 1], None, MULT,
                            )
                    ka()

            # one [128,128] transpose per q-block covers both heads
            for j in range(NB):
                tps = pp.tile([P, P], BF16, tag="mm", name=f"tps_{b}_{j}")
                nc.tensor.matmul(
                    tps, lhsT=att_j[j], rhs=id_sb,
                    is_transpose=True, start=True, stop=True,
                )
                nc.vector.tensor_copy(attT[:, j * P : (j + 1) * P], tps)

                # ======== partial projection for this q-block ========
                outst = outp.tile([P, C], BF16, tag="outst", name=f"outst_{b}_{j}")
                for n2 in range(2):
                    pps = pp.tile([P, 512], F32, tag="mm", name=f"pps_{b}_{j}_{n2}")
                    nc.tensor.matmul(
                        pps,
                        lhsT=attT[:, j * P : (j + 1) * P],
                        rhs=wp_bf[:, n2 * 512 : (n2 + 1) * 512],
                        start=True, stop=True,
                    )
                    if n2 == 0:
                        nc.vector.tensor_copy(outst[:, n2 * 512 : (n2 + 1) * 512], pps)
                    else:
                        nc.scalar.copy(outst[:, n2 * 512 : (n2 + 1) * 512], pps)
                # sync queue for stores; xq loads live on the gpsimd queue
                nc.sync.dma_start(out[b * T + j * P : b * T + (j + 1) * P, :], outst)
                ka()


def build_nc():
    nc = bacc.Bacc("TRN2", debug=False, enable_asserts=False, num_devices=NCORES)
    ins = {
        "xt": nc.dram_tensor("xt", [C, B * T], BF16, kind="ExternalInput").ap(),
        "wq": nc.dram_tensor("wq", [C, P], BF16, kind="ExternalInput").ap(),
        "wk": nc.dram_tensor("wk", [C, P], BF16, kind="ExternalInput").ap(),
        "wv": nc.dram_tensor("wv", [C, P], BF16, kind="ExternalInput").ap(),
        "wp": nc.dram_tensor("wp", [P, C], BF16, kind="ExternalInput").ap(),
        "bq": nc.dram_tensor("bq", [P, 1], F32, kind="ExternalInput").ap(),
        "bk": nc.dram_tensor("bk", [P, 1], F32, kind="ExternalInput").ap(),
        "maskadd": nc.dram_tensor("maskadd", [P, P], BF16, kind="ExternalInput").ap(),
        "ident": nc.dram_tensor("ident", [P, P], BF16, kind="ExternalInput").ap(),
    }
    outs = {"out": nc.dram_tensor("out", [B * T, C], BF16, kind="ExternalOutput").ap()}
    with tile.TileContext(nc) as tc:
        attention_body(tc, outs, ins)
    nc.compile()
    return nc


def make_in_maps(inputs, W_qkv, b_qkv, W_proj):
    x2 = np.asarray(inputs, np.float32).reshape(B * T, C)
    xtv = np.ascontiguousarray(x2.T).astype(ml_dtypes.bfloat16)
    W_qkv = np.asarray(W_qkv, np.float32)
    b_qkv = np.asarray(b_qkv, np.float32)
    W_proj = np.asarray(W_proj, np.float32)
    identv = np.eye(P, dtype=ml_dtypes.bfloat16)
    maskaddv = (np.triu(np.ones((P, P), np.float32), 1) * -1e9).astype(ml_dtypes.bfloat16)
    in_maps = []
    for cid in range(NCORES):
        s = cid * HPC * D
        in_maps.append({
            "xt": xtv,
            "wq": np.ascontiguousarray(W_qkv[:, s : s + P]).astype(ml_dtypes.bfloat16),
            "wk": np.ascontiguousarray(W_qkv[:, C + s : C + s + P]).astype(ml_dtypes.bfloat16),
            "wv": np.ascontiguousarray(W_qkv[:, 2 * C + s : 2 * C + s + P]).astype(ml_dtypes.bfloat16),
            "wp": np.ascontiguousarray(W_proj[s : s + P, :]).astype(ml_dtypes.bfloat16),
            "bq": np.ascontiguousarray(b_qkv[s : s + P].reshape(P, 1) * 0.125),
            "bk": np.ascontiguousarray(b_qkv[C + s : C + s + P].reshape(P, 1)),
            "maskadd": maskaddv,
            "ident": identv,
        })
    return in_maps


_NC_CACHE = {}


def run(inputs, W_qkv, b_qkv, W_proj, b_proj, trace=False, **kw):
    from concourse.bass_utils import run_bass_kernel_spmd

    if "nc" not in _NC_CACHE:
        _NC_CACHE["nc"] = build_nc()
    nc = _NC_CACHE["nc"]
    in_maps = make_in_maps(inputs, W_qkv, b_qkv, W_proj)
    res = run_bass_kernel_spmd(nc, in_maps, core_ids=list(range(NCORES)), trace=trace, **kw)
    acc = np.asarray(res.results[0]["out"], dtype=np.float32).copy()
    for cid in range(1, NCORES):
        acc += np.asarray(res.results[cid]["out"], dtype=np.float32)
    host_bias = np.asarray(b_qkv, np.float32)[2 * C :] @ np.asarray(W_proj, np.float32)
    host_bias = host_bias + np.asarray(b_proj, np.float32)
    outv = (acc + host_bias[None, :]).reshape(B, T, C).astype(np.float32)
    return outv, res


def kernel(inputs, W_qkv, b_qkv, W_proj, b_proj):
    outv, _ = run(inputs, W_qkv, b_qkv, W_proj, b_proj, trace=False)
    return outv


# revision 34
# speedup vs baseline: 1.0089x; 1.0089x over previous
"""Causal self-attention (B=4, T=2048, C=1024, H=16, D=64) on 8 trn2 NeuronCores.

Sharding: tensor-parallel over heads. Each core owns 2 heads:
  - computes Q^T/K^T/V for its heads from the (host-pretransposed) full x^T,
  - causal attention (transposed-S flash-style, softmax denominator via an
    augmented ones-column on V),
  - partial output projection with its 128 rows of W_proj.
Host sums the 8 partial projections and adds (b_v @ W_proj + b_proj).

The per-core program is identical (SPMD); only the weight-slice inputs differ.

v2 changes vs baseline:
  - causal mask applied as an extra -1e9 matmul into the diagonal S^T block
    (PE) instead of a gpsimd multiply after exp.
  - exp uses bias=-2 and writes at_sb in fp8e4m3 (faster AV weight loads,
    half the SBUF footprint).
  - AV accumulates 4 q-blocks per PSUM bank; one reciprocal per 4 blocks.
  - strips and AV interleaved per head so the scalar engine never starves.
  - qt/kt PSUM->SBUF copies moved off the scalar engine to DVE.
  - projection output copied/DMAed as bf16 (halves output traffic).
"""

import numpy as np
import ml_dtypes

import concourse.bass as bass
import concourse.bacc as bacc
import concourse.mybir as mybir
import concourse.tile as tile

B, T, C, H, D = 4, 2048, 1024, 16, 64
NCORES = 8
HPC = H // NCORES  # heads per core = 2
P = 128
NB = T // P  # 16 blocks of 128 per sequence
CK = C // P  # 8 contraction chunks for the projections

import os

F32 = mybir.dt.float32
F8 = mybir.dt.float8e4
BF16 = mybir.dt.bfloat16
AT_DT = F8 if os.environ.get("AT_FP8", "0") == "1" else BF16
ADD = mybir.AluOpType.add
MULT = mybir.AluOpType.mult
EXP = mybir.ActivationFunctionType.Exp
COPY = mybir.ActivationFunctionType.Copy


def _segments():
    """Per q-step c: the S^T strips that become complete at step c.
    Main strip (c, c, min(c+8,16)); tail strip (c-8, c, 16) for c>=8."""
    steps = []
    for c in range(NB):
        segs = [(c, c, min(c + 8, NB))]
        if c >= 8:
            segs.append((c - 8, c, NB))
        steps.append(segs)
    return steps


STEPS = _segments()


def _at_offsets():
    off = {}
    cur = 0
    for segs in STEPS:
        for (c, jlo, jhi) in segs:
            for j in range(jlo, jhi):
                off[(c, j)] = cur + (j - jlo) * P
            cur += (jhi - jlo) * P
    return off, cur


AT_OFF, AT_W = _at_offsets()  # AT_W = 136*128 = 17408


def attention_body(tc, outs, ins):
    """Tile kernel body. outs/ins are dicts of bass.APs (DRAM)."""
    nc = tc.nc
    xt = ins["xt"]  # [C, B*T] bf16 (x transposed, col = b*T + t)
    wq = ins["wq"]  # [C, 128] bf16
    wk = ins["wk"]  # [C, 128] bf16
    wv = ins["wv"]  # [C, 128] bf16
    wp = ins["wp"]          # [128, C] bf16
    bq = ins["bq"]          # [128, 1] f32 (prescaled by 0.125)
    bk = ins["bk"]          # [128, 1] f32
    maskadd = ins["maskadd"]  # [128, 128] bf16: -1e9 above diagonal (q<k)
    ident = ins["ident"]    # [128, 128] bf16 identity
    out = outs["out"]       # [B*T, C] bf16 partial projection output

    with (
        tc.tile_pool(name="consts", bufs=1) as consts,
        tc.tile_pool(name="xtp", bufs=4) as xtp,
        tc.tile_pool(name="qkp", bufs=3) as qkp,
        tc.tile_pool(name="vp", bufs=3) as vp,
        tc.tile_pool(name="atp", bufs=2) as atp,
        tc.tile_pool(name="smallp", bufs=4) as smallp,
        tc.tile_pool(name="outp", bufs=4) as outp,
        tc.tile_pool(name="pp", bufs=3, space="PSUM") as pp,
        tc.tile_pool(name="ppA", bufs=2, space="PSUM") as ppA,
        tc.tile_pool(name="ppV", bufs=1, space="PSUM") as ppV,
    ):
        # ---- constants ----
        wq_sb = consts.tile([P, CK, P], BF16, name="wq_sb")
        nc.sync.dma_start(wq_sb, wq.rearrange("(o p) m -> p o m", p=P))
        wk_sb = consts.tile([P, CK, P], BF16, name="wk_sb")
        nc.sync.dma_start(wk_sb, wk.rearrange("(o p) m -> p o m", p=P))
        wv_sb = consts.tile([P, CK, P], BF16, name="wv_sb")
        nc.sync.dma_start(wv_sb, wv.rearrange("(o p) m -> p o m", p=P))
        wp_bf = consts.tile([P, C], BF16, name="wp_bf")
        nc.sync.dma_start(wp_bf, wp)
        bq_sb = consts.tile([P, 1], F32, name="bq_sb")
        nc.gpsimd.dma_start(bq_sb, bq)
        bk_sb = consts.tile([P, 1], F32, name="bk_sb")
        nc.gpsimd.dma_start(bk_sb, bk)
        mask_sb = consts.tile([P, P], BF16, name="mask_sb")
        nc.gpsimd.dma_start(mask_sb, maskadd)
        id_sb = consts.tile([P, P], BF16, name="id_sb")
        nc.gpsimd.dma_start(id_sb, ident)
        ebias = consts.tile([P, 1], F32, name="ebias")
        nc.vector.memset(ebias, -2.0)

        keepalive = os.environ.get("KEEPALIVE", "0") == "1"

        def ka(n=2):
            # dependency-free LDWEIGHTS pulses keep the PE array active during
            # micro-stalls so the HAM clock gate stays at full rate
            if keepalive:
                for _ in range(n):
                    nc.tensor.ldweights(id_sb[:, 0:32])

        if os.environ.get("KEEPWARM", "1") == "1":
            # warm-up pre-roll: a continuous burst of dummy matmuls during the
            # initial DMA latency window un-throttles the HAM clock gate
            # before the first real stream starts (needs no DMA'd inputs)
            jw = consts.tile([P, P], BF16, name="jw")
            nc.vector.memset(jw, 0.0)
            jps = pp.tile([P, P], F32, tag="mm", name="jps")
            for _ in range(100):
                nc.tensor.matmul(jps, lhsT=jw, rhs=jw, start=True, stop=True)

        for b in range(B):
            # ======== QKV phase: Q^T, K^T (bf16), natural V (bf16, +ones col) ====
            qt = qkp.tile([P, T], BF16, tag="qt", name=f"qt_{b}")
            kt = qkp.tile([P, T], BF16, tag="kt", name=f"kt_{b}")
            vaug = vp.tile([P, NB, HPC, D + 1], BF16, tag="vaug", name=f"vaug_{b}")
            nc.gpsimd.memset(vaug[:, :, :, D:], 1.0)  # softmax-denominator column

            at_sbs = [
                atp.tile([P, AT_W], AT_DT, tag="at", name=f"at_{b}_{h}")
                for h in range(HPC)
            ]

            def do_strip(h, c, jlo, jhi):
                hs = h * D
                w = (jhi - jlo) * P
                sps = ppA.tile([P, 1024], F32, tag="sA", name=f"sps_{b}_{h}_{c}_{jlo}")
                lhs_k = kt[hs : hs + D, c * P : (c + 1) * P]
                col = 0
                while col < w:
                    n = min(512, w - col)
                    nc.tensor.matmul(
                        sps[:, col : col + n],
                        lhsT=lhs_k,
                        rhs=qt[hs : hs + D, jlo * P + col : jlo * P + col + n],
                        start=True, stop=True,
                        skip_group_check=True,
                    )
                    col += n
                if jlo == c:  # diagonal: additive causal mask via PE
                    nc.tensor.matmul(
                        sps[:, 0:P], lhsT=mask_sb, rhs=id_sb,
                        start=False, stop=True, skip_group_check=True,
                    )
                o = AT_OFF[(c, jlo)]
                nc.scalar.activation(at_sbs[h][:, o : o + w], sps[:, :w], EXP, bias=ebias)

            # steps whose qt/kt inputs are complete after quarter q4
            STEPS_AFTER_Q = {1: STEPS[0:1], 2: STEPS[1:5], 3: STEPS[5:NB]}

            for q4 in range(4):  # quarters of T (512 cols each)
                lo = q4 * 512
                xq = xtp.tile([P, CK, 512], BF16, tag="xq", name=f"xq_{b}_{q4}")
                nc.gpsimd.dma_start(
                    xq, xt[:, b * T + lo : b * T + lo + 512].rearrange("(o p) t -> p o t", p=P)
                )
                # V^T quarter first: its DVE cast hides behind the Q/K matmuls
                ps_v = pp.tile([P, 512], F32, tag="mm", name=f"psv_{b}_{q4}")
                for cc in range(CK):
                    nc.tensor.matmul(
                        ps_v, lhsT=wv_sb[:, cc], rhs=xq[:, cc],
                        start=(cc == 0), stop=(cc == CK - 1),
                    )
                vt = vp.tile([P, 512], BF16, tag="vt", name=f"vt_{b}_{q4}")
                nc.vector.tensor_copy(vt, ps_v)
                # Q^T quarter
                ps_q = pp.tile([P, 512], F32, tag="mm", name=f"psq_{b}_{q4}")
                for cc in range(CK):
                    nc.tensor.matmul(
                        ps_q, lhsT=wq_sb[:, cc], rhs=xq[:, cc],
                        start=(cc == 0), stop=(cc == CK - 1),
                    )
                # qt = psum * (1/sqrt(D)) + bq_prescaled  (DVE)
                nc.vector.tensor_scalar(
                    qt[:, lo : lo + 512], ps_q, 0.125, bq_sb, MULT, ADD
                )
                # transpose V^T -> natural V chunks (vt ready by now)
                # one accumulation group per head: mixing row-base 0/64
                # transposes in one PSUM group faults on hardware
                for h in range(HPC):
                    vtp = pp.tile([P, 4, D], BF16, tag="mm", name=f"vtp_{b}_{q4}_{h}")
                    for t4 in range(4):
                        nc.tensor.matmul(
                            vtp[:, t4],
                            lhsT=vt[h * D : (h + 1) * D, t4 * P : (t4 + 1) * P],
                            rhs=id_sb[h * D : (h + 1) * D, h * D : (h + 1) * D],
                            is_transpose=True,
                            start=(t4 == 0), stop=(t4 == 3),
                        )
                    nc.scalar.copy(vaug[:, q4 * 4 : (q4 + 1) * 4, h, 0:D], vtp)
                # K^T quarter
                ps_k = pp.tile([P, 512], F32, tag="mm", name=f"psk_{b}_{q4}")
                for cc in range(CK):
                    nc.tensor.matmul(
                        ps_k, lhsT=wk_sb[:, cc], rhs=xq[:, cc],
                        start=(cc == 0), stop=(cc == CK - 1),
                    )
                nc.vector.tensor_scalar(
                    kt[:, lo : lo + 512], ps_k, bk_sb, None, ADD
                )
                # S^T strips whose qt/kt inputs are now complete (both heads)
                for segs in STEPS_AFTER_Q.get(q4, []):
                    for (c, jlo, jhi) in segs:
                        for h in range(HPC):
                            do_strip(h, c, jlo, jhi)
                ka()

            # ======== attention per head ========
            attT = qkp.tile([P, T], BF16, tag="attT", name=f"attT_{b}")
            att_j = [
                smallp.tile([P, HPC * D], BF16, tag="attj", bufs=20, name=f"attj_{b}_{j}")
                for j in range(NB)
            ]
            # ---- AV phase: 4 q-blocks share one PSUM bank ----
            for h in range(HPC):
                hs = h * D
                at_sb = at_sbs[h]
                for j in range(NB):
                    jj = j % 4
                    if jj == 0:
                        avq = ppV.tile([P, 4, D + 1], F32, tag="avq", name=f"avq_{b}_{h}_{j}")
                    for c in range(j + 1):
                        o = AT_OFF[(c, j)]
                        nc.tensor.matmul(
                            avq[:, jj],
                            lhsT=at_sb[:, o : o + P],
                            rhs=vaug[:, c, h],
                            start=(c == 0), stop=(c == j),
                        )
                    if jj == 3:
                        r4 = smallp.tile([P, 4], F32, tag="r4", name=f"r4_{b}_{h}_{j}")
                        nc.vector.reciprocal(r4, avq[:, :, D])
                        for j2 in range(j - 3, j + 1):
                            nc.vector.tensor_scalar(
                                att_j[j2][:, hs : hs + D], avq[:, j2 % 4, 0:D],
                                r4[:, j2 % 4 : j2 % 4 + 1], None, MULT,
                            )
                    ka()

            # one [128,128] transpose per q-block covers both heads
            for j in range(NB):
                tps = pp.tile([P, P], BF16, tag="mm", name=f"tps_{b}_{j}")
                nc.tensor.matmul(
                    tps, lhsT=att_j[j], rhs=id_sb,
                    is_transpose=True, start=True, stop=True,
                )
                nc.vector.tensor_copy(attT[:, j * P : (j + 1) * P], tps)

                # ======== partial projection for this q-block ========
                outst = outp.tile([P, C], BF16, tag="outst", name=f"outst_{b}_{j}")
                for n2 in range(2):
                    pps = pp.tile([P, 512], F32, tag="mm", name=f"pps_{b}_{j}_{n2}")
                    nc.tensor.matmul(
                        pps,
                        lhsT=attT[:, j * P : (j + 1) * P],
                        rhs=wp_bf[:, n2 * 512 : (n2 + 1) * 512],
                        start=True, stop=True,
                    )
                    if n2 == 0:
                        nc.vector.tensor_copy(outst[:, n2 * 512 : (n2 + 1) * 512], pps)
                    else:
                        nc.scalar.copy(outst[:, n2 * 512 : (n2 + 1) * 512], pps)
                # sync queue for stores; xq loads live on the gpsimd queue
                nc.sync.dma_start(out[b * T + j * P : b * T + (j + 1) * P, :], outst)
                ka()


def build_nc():
    nc = bacc.Bacc("TRN2", debug=False, enable_asserts=False, num_devices=NCORES)
    ins = {
        "xt": nc.dram_tensor("xt", [C, B * T], BF16, kind="ExternalInput").ap(),
        "wq": nc.dram_tensor("wq", [C, P], BF16, kind="ExternalInput").ap(),
        "wk": nc.dram_tensor("wk", [C, P], BF16, kind="ExternalInput").ap(),
        "wv": nc.dram_tensor("wv", [C, P], BF16, kind="ExternalInput").ap(),
        "wp": nc.dram_tensor("wp", [P, C], BF16, kind="ExternalInput").ap(),
        "bq": nc.dram_tensor("bq", [P, 1], F32, kind="ExternalInput").ap(),
        "bk": nc.dram_tensor("bk", [P, 1], F32, kind="ExternalInput").ap(),
        "maskadd": nc.dram_tensor("maskadd", [P, P], BF16, kind="ExternalInput").ap(),
        "ident": nc.dram_tensor("ident", [P, P], BF16, kind="ExternalInput").ap(),
    }
    outs = {"out": nc.dram_tensor("out", [B * T, C], BF16, kind="ExternalOutput").ap()}
    with tile.TileContext(nc) as tc:
        attention_body(tc, outs, ins)
    nc.compile()
    return nc


def make_in_maps(inputs, W_qkv, b_qkv, W_proj):
    x2 = np.asarray(inputs, np.float32).reshape(B * T, C)
    xtv = np.ascontiguousarray(x2.T).astype(ml_dtypes.bfloat16)
    W_qkv = np.asarray(W_qkv, np.float32)
    b_qkv = np.asarray(b_qkv, np.float32)
    W_proj = np.asarray(W_proj, np.float32)
    identv = np.eye(P, dtype=ml_dtypes.bfloat16)
    maskaddv = (np.triu(np.ones((P, P), np.float32), 1) * -1e9).astype(ml_dtypes.bfloat16)
    in_maps = []
    for cid in range(NCORES):
        s = cid * HPC * D
        in_maps.append({
            "xt": xtv,
            "wq": np.ascontiguousarray(W_qkv[:, s : s + P]).astype(ml_dtypes.bfloat16),
            "wk": np.ascontiguousarray(W_qkv[:, C + s : C + s + P]).astype(ml_dtypes.bfloat16),
            "wv": np.ascontiguousarray(W_qkv[:, 2 * C + s : 2 * C + s + P]).astype(ml_dtypes.bfloat16),
            "wp": np.ascontiguousarray(W_proj[s : s + P, :]).astype(ml_dtypes.bfloat16),
            "bq": np.ascontiguousarray(b_qkv[s : s + P].reshape(P, 1) * 0.125),
            "bk": np.ascontiguousarray(b_qkv[C + s : C + s + P].reshape(P, 1)),
            "maskadd": maskaddv,
            "ident": identv,
        })
    return in_maps


_NC_CACHE = {}


def run(inputs, W_qkv, b_qkv, W_proj, b_proj, trace=False, **kw):
    from concourse.bass_utils import run_bass_kernel_spmd

    if "nc" not in _NC_CACHE:
        _NC_CACHE["nc"] = build_nc()
    nc = _NC_CACHE["nc"]
    in_maps = make_in_maps(inputs, W_qkv, b_qkv, W_proj)
    res = run_bass_kernel_spmd(nc, in_maps, core_ids=list(range(NCORES)), trace=trace, **kw)
    acc = np.asarray(res.results[0]["out"], dtype=np.float32).copy()
    for cid in range(1, NCORES):
        acc += np.asarray(res.results[cid]["out"], dtype=np.float32)
    host_bias = np.asarray(b_qkv, np.float32)[2 * C :] @ np.asarray(W_proj, np.float32)
    host_bias = host_bias + np.asarray(b_proj, np.float32)
    outv = (acc + host_bias[None, :]).reshape(B, T, C).astype(np.float32)
    return outv, res


def kernel(inputs, W_qkv, b_qkv, W_proj, b_proj):
    outv, _ = run(inputs, W_qkv, b_qkv, W_proj, b_proj, trace=False)
    return outv
